# revision 1
# baseline (speedup 1.0000x reference)
"""Causal multi-head attention on 8 Trainium2 cores (raw Bass).

Problem: x[4,2048,1024] @ W_qkv -> 16-head causal attention -> @ W_proj.
Sharding: core c handles batch b=c//2 and head-half c%2 (8 heads each).
Host pre-transposes x (feature-major xT) and pre-slices/scales weights;
each core computes its heads' contribution to out^T; host sums the two
half contributions per batch and adds b_proj.

Per-core pipeline (fp32r matmuls, fp32 PSUM):
  A1: qk^T[f,t] = w_qk^T @ x^T       (q rows pre-scaled by 1/sqrt(dk))
  A2: V[t,f] = x @ w_v (+bias); V_aug has a ones-column per head
  B:  per (head, q-chunk): S^T[k,q] = k^T.T @ q^T on causal blocks,
      P^T = exp(S^T) on ACT, triangle mask on diagonal 128-blocks (DVE),
      y_aug^T = V_aug^T @ P^T accumulated in PSUM (row 64 = softmax sums),
      reciprocal + K=1 replication matmul + DVE multiply to normalize.
      Odd heads staged at partitions 0:64 and DMA-shifted to 64:128.
  C:  out^T = w_proj^T @ y^T, DMA'd out transposed; host transposes back.

build_nc(t, reps) can replicate the whole pipeline `reps` times inside one
NEFF (serialized at rep boundaries) for wall-clock timing dilation.
"""

import contextlib
import math

import numpy as np

import concourse.bass as bass
import concourse.mybir as mybir
from concourse.bass_utils import run_bass_kernel_spmd

F32 = mybir.dt.float32
F32R = mybir.dt.float32r
ADD = mybir.AluOpType.add
MULT = mybir.AluOpType.mult
EXP = mybir.ActivationFunctionType.Exp
COPY = mybir.ActivationFunctionType.Copy

D_MODEL = 1024
D_K = 64
B, T = 4, 2048
NH = 8          # heads per core
KC = 8          # D_MODEL / 128
TQ = 512        # q-chunk width
N_CORES = 8


def build_nc(t=T, reps=1):
    tt_n = t // 128
    tc_n = t // TQ
    nc = bass.Bass(target_bir_lowering=False)

    xT_d = nc.dram_tensor("xT", [128, KC, t], F32R, kind="ExternalInput")
    wqk_d = nc.dram_tensor("wqk", [128, KC, 8, 128], F32R, kind="ExternalInput")
    wv_d = nc.dram_tensor("wv", [128, KC, 512], F32R, kind="ExternalInput")
    wproj_d = nc.dram_tensor("wproj", [128, 4, 1024], F32R, kind="ExternalInput")
    bqk_d = nc.dram_tensor("bqk", [128, 8], F32, kind="ExternalInput")
    bv_d = nc.dram_tensor("bv", [128, 512], F32, kind="ExternalInput")
    tri_d = nc.dram_tensor("tri", [128, 128], F32, kind="ExternalInput")
    ones_d = nc.dram_tensor("onesv", [128, 64], F32R, kind="ExternalInput")
    out_d = nc.dram_tensor("outT", [128, 8, t], F32, kind="ExternalOutput")

    # ---- schedule state ----
    prog = {"sync": [], "tensor": [], "vector": [], "scalar": []}
    cnt = {"pe": 0, "act": 0, "dve": 0}
    for _c in range(8):
        cnt[f"dma{_c}"] = 0
    last_wait = {e: {} for e in prog}
    bank_war = {}          # psum bank -> (sem, value): last consumer finished
    FUSE = {"tensor", "vector", "scalar"}

    def op(engine, fn, waits=(), incs=()):
        w = []
        for s, v in waits:
            if v <= 0 or last_wait[engine].get(s, -1) >= v:
                continue
            last_wait[engine][s] = v
            w.append((s, v))
        prog[engine].append((fn, w, list(incs), engine in FUSE))
        for s, a in incs:
            cnt[s] += a

    NDMA = 8
    dma_rr = [0]

    def dma(dst, src, waits=()):
        ch = dma_rr[0] % NDMA
        dma_rr[0] += 1
        sem = f"dma{ch}"
        w = [(sem, cnt[sem])] + list(waits)   # chain within channel
        op("sync", lambda e, d=dst, s=src: e.dma_start(d, s),
           w, [(sem, 16)])
        return (sem, cnt[sem])

    stack = contextlib.ExitStack()
    sb = lambda name, shape, dt: stack.enter_context(
        nc.sbuf_tensor(name, shape, dt))

    # persistent region
    qk_sb = sb("qk", [128, 8, t], F32R)
    v_sb = sb("vsb", [128, tt_n, 8, 65], F32R)
    bqk_sb = sb("bqk_sb", [128, 8], F32)
    bv_sb = sb("bv_sb", [128, 512], F32)
    tri_sb = sb("tri_sb", [128, 128], F32)
    ones_sb = sb("ones_sb", [128, 64], F32R)
    psum = stack.enter_context(nc.psum_tensor("ps", [128, 8, 512], F32))

    with contextlib.ExitStack() as semstack:
        semstack.enter_context(nc.allow_low_precision(
            reason="fp32r rounding on matmul operands is intentional"))
        sems = {}
        for _nm in ["pe", "act", "dve"] + [f"dma{_c}" for _c in range(8)]:
            sems[_nm] = semstack.enter_context(nc.semaphore(_nm + "_sem"))

        bqk_ret = dma(bqk_sb.ap(), bqk_d[:])
        bv_ret = dma(bv_sb.ap(), bv_d[:])
        tri_ret = dma(tri_sb.ap(), tri_d[:])
        ones_ret = dma(ones_sb.ap(), ones_d[:])

        # ones column of V_aug via DVE (x*0 + 1)
        op("vector",
           lambda e: e.tensor_scalar(
               v_sb.ap()[:, :, :, 64:65],
               bv_sb.ap()[:, 0:tt_n * 8].rearrange(
                   "p (a b c) -> p a b c", a=tt_n, b=8),
               0.0, 1.0, MULT, mybir.AluOpType.add),
           [bv_ret], [("dve", 1)])
        vones_ret = ("dve", cnt["dve"])

        # phase-A region (aliased by phase-B/C tiles; reps serialize fully)
        xa = nc.sbuf_tensor("xT_sb", [128, KC, t], F32R)
        xT_sb = xa.__enter__()
        wqa = nc.sbuf_tensor("wqg", [128, 2, KC, 4, 128], F32R)
        wqg = wqa.__enter__()
        wva = nc.sbuf_tensor("wv_buf", [128, 4, 512], F32R)
        wv_buf = wva.__enter__()
        a_tiles_end = [xa, wqa, wva]

        # reserve phase-B/C tiles now (addresses alias the A region; safe
        # because within a rep B starts only after A's last read, and reps
        # are serialized at the boundary)
        for _a in reversed(a_tiles_end):
            pass  # keep handles; exit later

        first_pv = [True]
        pair_war = {0: 0, 1: 0}
        slot_war = {0: 0, 1: 0}
        ybank_war = {}
        rbank_war = {}
        rsb_war = {}
        ysbt_war = {}
        wqg_last = {}           # kc -> pe cnt of last MM reading wqg[kc]
        pending_tail = []

        def _make_tail(rb, i, recip_done, out_ap, yb, h, g):
            # returns op-tuples for (rep matmul, normalize) of iteration i
            def emit():
                w = [("dve", recip_done), ones_ret]
                if rb in rbank_war:
                    w.append(("dve", rbank_war[rb]))
                op("tensor",
                   lambda e, rb=rb, buf=i % 2: e.matmul(
                       psum.ap()[0:64, rb],
                       ones_sb.ap()[64:65, :],
                       rsb.ap()[64:65, buf],
                       start=True, stop=True),
                   w, [("pe", 1)])
                rep_done = cnt["pe"]
                rsb_war[i % 2] = rep_done
                w = [("pe", rep_done), ("dve", ybank_war[yb])]
                if h % 2 == 1 and (g % 2) in ysbt_war:
                    w.append(ysbt_war[g % 2])
                op("vector",
                   lambda e, o=out_ap, rb=rb, buf=i % 2:
                       e.tensor_tensor(
                           o, yun.ap()[0:64, buf].bitcast(F32),
                           psum.ap()[0:64, rb], MULT),
                   w, [("dve", 1)])
                rbank_war[rb] = cnt["dve"]
            return emit
        c_copy = {}
        c_dma = {}
        out_seq = [0]
        rep_gate = []
        b_alloc = [None]

        for rep in range(reps):
            # ---- phase A1 ----
            xT_done = {}
            wqg_dma = {0: {}, 1: {}}
            if True:
                for kc in range(KC):
                    war = list(rep_gate)
                    if (0, kc) in wqg_last:
                        war.append(("pe", wqg_last[(0, kc)]))
                    wqg_dma[0][kc] = dma(
                        wqg.ap()[:, 0, kc], wqk_d[:, kc, 0:4], war)
                    xT_done[kc] = dma(
                        xT_sb.ap()[:, kc], xT_d[:, kc], list(rep_gate))
                for kc in range(KC):
                    war = list(rep_gate)
                    if (1, kc) in wqg_last:
                        war.append(("pe", wqg_last[(1, kc)]))
                    wqg_dma[1][kc] = dma(
                        wqg.ap()[:, 1, kc], wqk_d[:, kc, 4:8], war)
            for grp in range(2):
                for tc in range(tc_n):
                    for kc in range(KC):
                        for ftl in range(4):
                            bank = (tc % 2) * 4 + ftl
                            w = []
                            if ftl == 0:
                                w = [wqg_dma[grp][kc], xT_done[kc]] + rep_gate
                            if kc == 0 and bank in bank_war:
                                w.append(bank_war.pop(bank))
                            op("tensor",
                               lambda e, b=bank, g_=grp, k=kc, f=ftl, tc_=tc:
                                   e.matmul(
                                       psum.ap()[:, b],
                                       wqg.ap()[:, g_, k, f],
                                       xT_sb.ap()[:, k,
                                                  tc_ * TQ:(tc_ + 1) * TQ],
                                       start=(k == 0), stop=(k == KC - 1)),
                               w, [("pe", 1)] if ftl == 3 else [])
                        if tc == tc_n - 1:
                            wqg_last[(grp, kc)] = cnt["pe"]
                    grp_done = cnt["pe"]
                    for ftl in range(4):
                        ft = grp * 4 + ftl
                        bk = (tc % 2) * 4 + ftl
                        op("vector",
                           lambda e, b=bk, f=ft, tc_=tc:
                               e.tensor_scalar(
                                   qk_sb.ap()[:, f, tc_ * TQ:(tc_ + 1) * TQ],
                                   psum.ap()[:, b],
                                   bqk_sb.ap()[:, f:f + 1], None, ADD),
                           [("pe", grp_done), bqk_ret], [("dve", 1)])
                        bank_war[bk] = ("dve", cnt["dve"])
            a1_copies = cnt["dve"]

            # ---- phase A2 (wv streamed JIT through 4-slot buffer) ----
            tt_groups = [list(range(i, min(i + 4, tt_n)))
                         for i in range(0, tt_n, 4)]
            wv_seq = 0
            a2_kc_done = {}
            for tg, tts in enumerate(tt_groups):
                for kc in range(KC):
                    slot = wv_seq % 4
                    war = list(rep_gate)
                    if wv_seq - 4 >= 0:
                        war.append(("pe", a2_kc_done[wv_seq - 4]))
                    nd = dma(wv_buf.ap()[:, slot], wv_d[:, kc], war)
                    for j, tt in enumerate(tts):
                        bank = (tg % 2) * 4 + j
                        w = [nd] if j == 0 else []
                        if kc == 0 and bank in bank_war:
                            w.append(bank_war.pop(bank))
                        op("tensor",
                           lambda e, b=bank, s=slot, k=kc, tt_=tt:
                               e.matmul(
                                   psum.ap()[:, b],
                                   xT_sb.ap()[:, k, tt_ * 128:(tt_ + 1) * 128],
                                   wv_buf.ap()[:, s],
                                   start=(k == 0), stop=(k == KC - 1)),
                           w, [("pe", 1)] if j == len(tts) - 1 else [])
                    a2_kc_done[wv_seq] = cnt["pe"]
                    wv_seq += 1
                grp_done = cnt["pe"]
                for j, tt in enumerate(tts):
                    bk = (tg % 2) * 4 + j
                    op("vector",
                       lambda e, b=bk, tt_=tt:
                           e.tensor_tensor(
                               v_sb.ap()[:, tt_, :, 0:64],
                               psum.ap()[:, b], bv_sb.ap()[:], ADD),
                       [("pe", grp_done), bv_ret], [("dve", 1)])
                    bank_war[bk] = ("dve", cnt["dve"])
            a2_copies = cnt["dve"]
            a2_pe_done = cnt["pe"]

            if b_alloc[0] is None:
                for _a in reversed(a_tiles_end):
                    _a.__exit__(None, None, None)
                ysb = sb("ysb", [128, 4, t], F32R)
                ysbt = sb("ysbt", [64, 2, t], F32R)
                pt_sb = sb("pt", [128, 4, 512], F32R)
                yun = sb("yun", [64, 2, 512], F32R)
                rsb = sb("rsb", [65, 2, 512], F32R)
                osb = sb("osb", [128, 8, 512], F32)
                wproj_sb = sb("wproj_sb", [128, 4, 1024], F32R)
                b_alloc[0] = (ysb, ysbt, pt_sb, yun, rsb, osb, wproj_sb)
            else:
                ysb, ysbt, pt_sb, yun, rsb, osb, wproj_sb = b_alloc[0]

            wproj_dma = dma(wproj_sb.ap(), wproj_d[:], [("pe", a2_pe_done)])

            # ---- phase B ----
            for h in range(NH):
                g = h // 2
                qrow = (h % 2) * 64
                qf, kf = g, 4 + g
                for qc in range(tc_n):
                    i = h * tc_n + qc
                    yb = 4 + i % 2
                    rb = 6 + i % 2
                    nkt = 4 * qc + 4
                    npairs = 2 * qc + 2

                    def s_mm(kt, bank, qrow=qrow, kf=kf, qf=qf, qc=qc):
                        r = kt - 4 * qc
                        off = max(0, r * 128)
                        n = TQ - off
                        return lambda e, kt=kt, b=bank, off=off, n=n: \
                            e.matmul(
                                psum.ap()[:, b, off:off + n],
                                qk_sb.ap()[qrow:qrow + 64, kf,
                                           kt * 128:(kt + 1) * 128],
                                qk_sb.ap()[qrow:qrow + 64, qf,
                                           qc * TQ + off:qc * TQ + off + n],
                                start=True, stop=True)

                    def pv_mm(kt, slot, start, stop, h=h, qc=qc, yb=yb):
                        r = kt - 4 * qc
                        off = max(0, r * 128)
                        n = TQ - off
                        return lambda e, kt=kt, s=slot, off=off, n=n, \
                            st=start, sp=stop: e.matmul(
                                psum.ap()[0:65, yb, off:off + n],
                                v_sb.ap()[:, kt, h, :],
                                pt_sb.ap()[:, s, off:off + n],
                                start=st, stop=sp)

                    s_done = {}
                    pt_ready = {}

                    for p in range(npairs):
                        pg = p % 2
                        kts = (2 * p, 2 * p + 1)
                        banks = (pg * 2, pg * 2 + 1)
                        w = [("act", pair_war[pg]), ("dve", a1_copies)]
                        if p == 1 and pending_tail:
                            for _t in pending_tail:
                                _t()
                            pending_tail.clear()
                        for bq in banks:
                            if bq in bank_war:
                                w.append(bank_war.pop(bq))
                        op("tensor", s_mm(kts[0], banks[0]), w, [])
                        op("tensor", s_mm(kts[1], banks[1]), [], [("pe", 1)])
                        s_done[p] = cnt["pe"]
                        if p >= 1:
                            pp = p - 1
                            w = [pt_ready[pp]]
                            if first_pv[0]:
                                w += [vones_ret, ("dve", a2_copies)]
                                first_pv[0] = False
                            if pp == 0 and yb in ybank_war:
                                w.append(("dve", ybank_war[yb]))
                            op("tensor",
                               pv_mm(2 * pp, (pp % 2) * 2,
                                     2 * pp == 0, False), w, [])
                            op("tensor",
                               pv_mm(2 * pp + 1, (pp % 2) * 2 + 1, False,
                                     2 * pp + 1 == nkt - 1),
                               [], [("pe", 1)])
                            slot_war[pp % 2] = cnt["pe"]
                        # exp over the whole pair (dead regions of diagonal
                        # blocks hold bounded garbage; PV never reads them)
                        diag = (kts[1] - 4 * qc) >= 0
                        off0 = max(0, (kts[0] - 4 * qc)) * 128
                        w = [("pe", s_done[p]), ("pe", slot_war[pg])]
                        op("scalar",
                           lambda e, bq=banks[0], s=pg * 2, o=off0:
                               e.activation(
                                   pt_sb.ap()[:, s:s + 2]
                                       .rearrange("p a b -> p (a b)")
                                       [:, o:2 * TQ],
                                   psum.ap()[:, bq:bq + 2]
                                       .rearrange("p a b -> p (a b)")
                                       [:, o:2 * TQ],
                                   EXP),
                           w, [("act", 1)])
                        pair_war[pg] = cnt["act"]
                        pt_ready[p] = ("act", cnt["act"])
                        if diag:
                            for j in (0, 1):
                                r = kts[j] - 4 * qc
                                op("vector",
                                   lambda e, s=pg * 2 + j, r=r:
                                       e.tensor_tensor(
                                           pt_sb.ap()[:, s,
                                                      r * 128:r * 128 + 128],
                                           pt_sb.ap()[:, s,
                                                      r * 128:r * 128 + 128],
                                           tri_sb.ap()[:], MULT),
                                   [("act", pt_ready[p][1]), tri_ret],
                                   [("dve", 1)] if j == 1 else [])
                            pt_ready[p] = ("dve", cnt["dve"])

                    pp = npairs - 1
                    w = [pt_ready[pp]]
                    if pp == 0:
                        if first_pv[0]:
                            w += [vones_ret, ("dve", a2_copies)]
                            first_pv[0] = False
                        if yb in ybank_war:
                            w.append(("dve", ybank_war[yb]))
                    op("tensor", pv_mm(2 * pp, (pp % 2) * 2,
                                       2 * pp == 0, False), w, [])
                    op("tensor", pv_mm(2 * pp + 1, (pp % 2) * 2 + 1,
                                       False, True), [], [("pe", 1)])
                    slot_war[pp % 2] = cnt["pe"]
                    pv_all = cnt["pe"]

                    w = [("pe", pv_all)]
                    if i % 2 in rsb_war:
                        w.append(("pe", rsb_war[i % 2]))
                    op("vector",
                       lambda e, yb=yb, buf=i % 2: e.reciprocal(
                           rsb.ap()[64:65, buf], psum.ap()[64:65, yb]),
                       w, [("dve", 1)])
                    recip_done = cnt["dve"]
                    op("vector",
                       lambda e, yb=yb, buf=i % 2: e.tensor_copy(
                           yun.ap()[0:64, buf].bitcast(F32),
                           psum.ap()[0:64, yb]),
                       [], [("dve", 1)])
                    ybank_war[yb] = cnt["dve"]
                    if h % 2 == 0:
                        out_ap = ysb.ap()[0:64, g, qc * TQ:(qc + 1) * TQ]
                    else:
                        out_ap = ysbt.ap()[0:64, g % 2,
                                           qc * TQ:(qc + 1) * TQ]
                    pending_tail.append(_make_tail(
                        rb, i, recip_done, out_ap, yb, h, g))
                if h % 2 == 1:
                    for _t in pending_tail:
                        _t()
                    pending_tail.clear()
                    nd = dma(ysb.ap()[64:128, g], ysbt.ap()[0:64, g % 2],
                             [("dve", cnt["dve"])])
                    ysbt_war[g % 2] = nd
            for _t in pending_tail:
                _t()
            pending_tail.clear()
            b_dve_done = cnt["dve"]
            b_act_done = cnt["act"]
            shift_rets = [ysbt_war[k] for k in ysbt_war]

            # ---- phase C ----
            for tc in range(tc_n):
                for ft in range(8):
                    j = out_seq[0]
                    bank = j % 4
                    w = [("dve", b_dve_done), wproj_dma,
                         ("act", b_act_done)] + shift_rets
                    if j >= 4:
                        w.append(("act", c_copy[j - 4]))
                    for gg in range(4):
                        op("tensor",
                           lambda e, bk=bank, g_=gg, f=ft, tc_=tc: e.matmul(
                               psum.ap()[:, bk],
                               wproj_sb.ap()[:, g_, f * 128:(f + 1) * 128],
                               ysb.ap()[:, g_, tc_ * TQ:(tc_ + 1) * TQ],
                               start=(g_ == 0), stop=(g_ == 3)),
                           w if gg == 0 else [],
                           [("pe", 1)] if gg == 3 else [])
                    mm_done = cnt["pe"]
                    w = [("pe", mm_done)]
                    if j >= 8:
                        w.append(c_dma[j - 8])
                    op("scalar",
                       lambda e, bk=bank, ob=j % 8: e.activation(
                           osb.ap()[:, ob], psum.ap()[:, bk], COPY),
                       w, [("act", 1)])
                    c_copy[j] = cnt["act"]
                    bank_war[bank] = ("act", cnt["act"])
                    c_dma[j] = dma(
                        out_d[:, ft, tc * TQ:(tc + 1) * TQ],
                        osb.ap()[:, j % 8],
                        [("act", c_copy[j])])
                    out_seq[0] += 1
            rep_gate = [("act", c_copy[out_seq[0] - 1]), c_dma[out_seq[0] - 1]]
            # seed psum WARs for next rep's A phase (banks 2,3 were last read
            # by B exps; 4..7 by B's recip/copy/norm)
            bank_war.setdefault(2, ("act", b_act_done))
            bank_war.setdefault(3, ("act", b_act_done))
            for bk in (4, 5):
                bank_war.setdefault(bk, ("dve", ybank_war.get(bk, 0)))
            for bk in (6, 7):
                bank_war.setdefault(bk, ("dve", rbank_war.get(bk, 0)))

        # ---- emit ----
        with nc.Block() as block:
            def emitter(name):
                def run(eng):
                    for fn, waits, incs, fuse in prog[name]:
                        pre = waits[1:] if (fuse and waits) else waits
                        for s, v in pre:
                            eng.wait_ge(sems[s], v)
                        ins = fn(eng)
                        if fuse and waits:
                            s, v = waits[0]
                            ins.wait_op(sems[s], v, "sem-ge")
                        for s, a in incs:
                            ins.then_inc(sems[s], a)
                return run
            block.sync(emitter("sync"))
            block.tensor(emitter("tensor"))
            block.vector(emitter("vector"))
            block.scalar(emitter("scalar"))

    stack.close()
    return nc


# ---------------------------------------------------------------------------

def host_prep(x, W_qkv, b_qkv, W_proj, b_proj, t=T):
    scale = 1.0 / math.sqrt(D_K)
    x = np.asarray(x, np.float32)
    W_qkv = np.asarray(W_qkv, np.float32)
    b_qkv = np.asarray(b_qkv, np.float32)
    W_proj = np.asarray(W_proj, np.float32)

    tri = (np.arange(128)[None, :] >= np.arange(128)[:, None]) \
        .astype(np.float32)
    onesv = np.ones((128, 64), np.float32)

    in_maps = []
    for c in range(N_CORES):
        b = c // 2
        f0 = (c % 2) * 512
        xT = np.ascontiguousarray(
            x[b, :t].T.reshape(KC, 128, t).transpose(1, 0, 2))
        wq = W_qkv[:, f0:f0 + 512] * scale
        wk = W_qkv[:, D_MODEL + f0:D_MODEL + f0 + 512]
        wqk = np.concatenate([wq, wk], axis=1)
        wqk = np.ascontiguousarray(
            wqk.reshape(KC, 128, 8, 128).transpose(1, 0, 2, 3))
        wv = W_qkv[:, 2 * D_MODEL + f0:2 * D_MODEL + f0 + 512]
        wv = np.ascontiguousarray(
            wv.reshape(KC, 128, 512).transpose(1, 0, 2))
        bq = b_qkv[f0:f0 + 512] * scale
        bk = b_qkv[D_MODEL + f0:D_MODEL + f0 + 512]
        bqk = np.ascontiguousarray(
            np.concatenate([bq, bk]).reshape(8, 128).T)
        bv = b_qkv[2 * D_MODEL + f0:2 * D_MODEL + f0 + 512]
        bv_rep = np.broadcast_to(bv, (128, 512)).copy()
        wp = W_proj[f0:f0 + 512]
        wp = np.ascontiguousarray(
            wp.reshape(4, 128, 1024).transpose(1, 0, 2))
        in_maps.append({
            "xT": xT, "wqk": wqk, "wv": wv, "wproj": wp,
            "bqk": bqk, "bv": bv_rep, "tri": tri, "onesv": onesv,
        })
    return in_maps


def host_gather(results, b_proj, t=T):
    b_proj = np.asarray(b_proj, np.float32)
    out = np.empty((B, t, D_MODEL), np.float32)
    for b in range(B):
        acc = None
        for half in range(2):
            r = results[2 * b + half]["outT"]
            oT = r.transpose(1, 0, 2).reshape(D_MODEL, t)
            acc = oT if acc is None else acc + oT
        out[b] = acc.T + b_proj
    return out


_NC_CACHE = {}


def kernel(x, W_qkv, b_qkv, W_proj, b_proj):
    if T not in _NC_CACHE:
        _NC_CACHE[T] = build_nc(T)
    nc = _NC_CACHE[T]
    in_maps = host_prep(x, W_qkv, b_qkv, W_proj, b_proj)
    res = run_bass_kernel_spmd(nc, in_maps, core_ids=list(range(N_CORES)))
    return host_gather(res.results, b_proj)

